# revision 8
# baseline (speedup 1.0000x reference)
"""Chamfer distance L2 kernel for Trainium2 (8 NeuronCores, SPMD).

Problem: xyz1 [B=8, N=8192, 3] f32, xyz2 [B=8, M=8192, 3] f32.
  d[b,n,m] = ||xyz1[b,n] - xyz2[b,m]||^2
  out = mean_bn(min_m d) + mean_bm(min_n d)   (scalar f32)

Sharding: data-parallel over batch; core b handles batch b (its full
8192x8192 distance block, both min directions). The two scalar means are
combined on the host from per-point min vectors (tiny: 2*[128,64] f32
per core).

Device algorithm (per core):
  The distance matrix is produced tile-by-tile straight into PSUM by the
  tensor engine and min-reduced on the fly; it never touches HBM.
  d = sq1[n] + sq2[m] - 2*x1.x2 is computed as a single K=18 bf16 matmul
  using a hi/lo split of the coordinates (bilinear expansion, bf16
  products are exact in fp32) and a 3-way bf16 split of the squared
  norms, so PSUM holds d to ~1e-6 absolute despite bf16 inputs. This
  runs the PE at full rate (1 col/cycle) instead of the 4x fp32 penalty.
  Min over the free axis is taken by the vector engine directly from
  PSUM for part of the tiles; the scalar engine converts the rest to
  bf16 in SBUF where the DVE combines them at 2 results/cycle (the PSUM
  read ports are the bottleneck: 1 fp32/cycle/lane per engine).
  A second matmul pass with the roles of xyz1/xyz2 swapped produces the
  min over n (cross-partition mins are not available on the DVE).
"""

import numpy as np

try:
    from concourse import bacc, mybir
except ImportError:  # path fallback
    import sys

    sys.path.insert(0, "/opt/trn_rl_repo")
    from concourse import bacc, mybir

import ml_dtypes
import concourse.bass as bass
import concourse.tile as tile
from concourse.bass_utils import run_bass_kernel_spmd

BF16 = ml_dtypes.bfloat16

# ---- problem constants (hardcoded per spec) ----
B = 8
NPTS = 8192  # N == M
NCORES = 8

# ---- kernel tiling config ----
P = 128            # partition tile (rows of output block)
FREE = 512         # matmul free dim = one PSUM bank of fp32
GROUP = 4          # PSUM banks per drain group -> [128, 2048] drains
KROWS = 18         # contraction rows of the split matmul
# Of the NG drain groups per n-tile, this many are drained via
# ACT copy to bf16 SBUF + DVE combine instead of direct DVE reduce.
ACT_GROUPS = 3
# Alternate PE row groups (base partition 0/32) between n-tiles so the
# next tile's LDWEIGHTS overlaps in-flight matmuls. NOTE: measured to
# crash the device (NRT_EXEC_UNIT_UNRECOVERABLE) — keep False.
ALT_ROWGROUPS = False

F32 = mybir.dt.float32
F16_DT = mybir.dt.float16
BF16_DT = mybir.dt.bfloat16
AX = mybir.AxisListType.X
MIN = mybir.AluOpType.min

_cache = {}


def _split3_f64(v):
    """fp64 vector -> 3 bf16 rows summing to v up to ~2^-25 relative."""
    h = v.astype(BF16)
    r = v - h.astype(np.float64)
    m = r.astype(BF16)
    r2 = r - m.astype(np.float64)
    l = r2.astype(BF16)
    return h, m, l


def _prep_side(xyz):
    """[N,3] f32 -> (W [18,N] bf16, Mv [18,N] bf16).

    W rows:  [h(3), l(3), h(3), l(3), s0, s1, s2, 1, 1, 1]
    Mv rows: [-2h'(3), -2h'(3), -2l'(3), -2l'(3), 1, 1, 1, s0', s1', s2']
    so that sum_k W[k,n]*Mv[k,m] = s[n] + s'[m] - 2*xt[n].xt'[m]
    where xt = hi+lo is the bf16-split representation of the points and
    s = |xt|^2 split into 3 bf16 rows.
    """
    x = np.ascontiguousarray(xyz.T).astype(np.float32)  # [3, N]
    h = x.astype(BF16)
    l = (x - h.astype(np.float32)).astype(BF16)
    hf = h.astype(np.float64)
    lf = l.astype(np.float64)
    xt = hf + lf
    sq = (xt * xt).sum(axis=0)  # [N] fp64, exact-ish
    s0, s1, s2 = _split3_f64(sq)
    one = np.ones((1, x.shape[1]), BF16)
    W = np.concatenate(
        [h, l, h, l, s0[None], s1[None], s2[None], one, one, one], axis=0
    )
    h2 = (-2.0 * hf).astype(BF16)  # exact scaling
    l2 = (-2.0 * lf).astype(BF16)
    Mv = np.concatenate(
        [h2, h2, l2, l2, one, one, one, s0[None], s1[None], s2[None]], axis=0
    )
    return np.ascontiguousarray(W), np.ascontiguousarray(Mv)


# tensor_tensor_reduce measured to crash the device (NRT unrecoverable);
# keep False unless re-validated.
USE_TTR = False


def _combine_act_groups(nc, scratch, sbs, out_col, gw):
    """Min-combine the ACT-copied fp16 tiles into one fp32 column [P,1].

    Pairwise fp16 tensor_tensor folds (2 results/cycle) down to one
    tile, halving folds down to FREE wide, then one tensor_reduce.
    """
    sbs = list(sbs)
    if len(sbs) == 1:
        nc.vector.tensor_reduce(out_col, sbs[0][:], axis=AX, op=MIN)
        return
    if USE_TTR:
        while len(sbs) > 2:
            a = sbs.pop(0)
            b = sbs.pop(0)
            u = scratch.tile([P, gw], F16_DT, tag="fold")
            nc.vector.tensor_tensor(u[:], a[:], b[:], MIN)
            sbs.append(u)
        a, b = sbs
        trash = scratch.tile([P, gw], F16_DT, tag="trash")
        nc.vector.tensor_tensor_reduce(
            trash[:], a[:], b[:], 1.0, 3.0e38, MIN, MIN, out_col
        )
        return
    while len(sbs) > 1:
        a = sbs.pop(0)
        b = sbs.pop(0)
        u = scratch.tile([P, gw], F16_DT, tag="fold")
        nc.vector.tensor_tensor(u[:], a[:], b[:], MIN)
        sbs.append(u)
    v = sbs[0]
    w = gw
    while w > FREE:
        half = w // 2
        nxt = scratch.tile([P, half], F16_DT, tag=f"h{half}")
        nc.vector.tensor_tensor(nxt[:], v[:, 0:half], v[:, half:w], MIN)
        v = nxt
        w = half
    nc.vector.tensor_reduce(out_col, v[:, 0:w], axis=AX, op=MIN)


def _matrix_pass(tc, pools, wtile, mtile, mins_tile, npts_n, npts_m):
    """One direction: for each n-tile of 128 rows, min over all m."""
    nc = tc.nc
    psum, scratch, pmpool = pools
    nt_count = npts_n // P
    ng_count = npts_m // (FREE * GROUP)
    gw = FREE * GROUP  # group width in elements
    act_groups = min(ACT_GROUPS, ng_count - 1)
    ndirect = ng_count - act_groups
    ncols = ndirect + (1 if act_groups else 0)

    for nt in range(nt_count):
        bp = 32 * (nt % 2) if ALT_ROWGROUPS else 0
        wap = wtile[bp : bp + KROWS, nt * P : (nt + 1) * P]
        pmt = pmpool.tile([P, ncols], F32, tag="pmt")
        sbs = []
        for g in range(ng_count):
            ps = psum.tile([P, gw], F32, tag="ps")
            for c in range(GROUP):
                mc = g * GROUP + c
                nc.tensor.matmul(
                    ps[:, c * FREE : (c + 1) * FREE],
                    wap,
                    mtile[bp : bp + KROWS, mc * FREE : (mc + 1) * FREE],
                    start=True,
                    stop=True,
                )
            if g < ndirect:
                # direct drain: DVE min-reduce straight from PSUM
                nc.vector.tensor_reduce(pmt[:, g : g + 1], ps[:], axis=AX, op=MIN)
            else:
                # ACT converts PSUM -> SBUF bf16 (1 elem/cycle @1.2GHz)
                sb = scratch.tile([P, gw], F16_DT, tag="sb")
                nc.scalar.copy(sb[:], ps[:])
                sbs.append(sb)
        if sbs:
            _combine_act_groups(nc, scratch, sbs, pmt[:, ndirect : ndirect + 1], gw)
        nc.vector.tensor_reduce(mins_tile[:, nt : nt + 1], pmt[:], axis=AX, op=MIN)


def _build_body(tc, aps, npts):
    nc = tc.nc
    wa, mb_, wb, ma, o1, o2 = aps
    from contextlib import ExitStack

    ctx = ExitStack()
    inp = ctx.enter_context(tc.tile_pool(name="inp", bufs=1))
    psum = ctx.enter_context(tc.tile_pool(name="psum", bufs=2, space="PSUM"))
    scratch = ctx.enter_context(tc.tile_pool(name="scratch", bufs=4))
    pmpool = ctx.enter_context(tc.tile_pool(name="pm", bufs=3))
    minspool = ctx.enter_context(tc.tile_pool(name="mins", bufs=1))

    nt_count = npts // P

    def load_dual(ap_dram, name):
        rows = (32 + KROWS) if ALT_ROWGROUPS else KROWS
        t = inp.tile([rows, npts], BF16_DT, tag=name)
        nc.sync.dma_start(t[0:KROWS, :], ap_dram[:])
        if ALT_ROWGROUPS:
            nc.sync.dma_start(t[32 : 32 + KROWS, :], ap_dram[:])
        return t

    twa = load_dual(wa, "twa")
    tmb = load_dual(mb_, "tmb")
    twb = load_dual(wb, "twb")
    tma = load_dual(ma, "tma")

    mins1 = minspool.tile([P, nt_count], F32, tag="mins1")
    mins2 = minspool.tile([P, nt_count], F32, tag="mins2")

    pools = (psum, scratch, pmpool)
    _matrix_pass(tc, pools, twa, tmb, mins1, npts, npts)
    _matrix_pass(tc, pools, twb, tma, mins2, npts, npts)

    nc.sync.dma_start(o1[:], mins1[:])
    nc.sync.dma_start(o2[:], mins2[:])
    ctx.close()


def build_nc(npts=NPTS, ncores=NCORES):
    nc = bacc.Bacc(
        "TRN2", target_bir_lowering=False, debug=False, num_devices=ncores
    )
    nt_count = npts // P
    wa = nc.dram_tensor("wa", [KROWS, npts], BF16_DT, kind="ExternalInput").ap()
    mb_ = nc.dram_tensor("mb", [KROWS, npts], BF16_DT, kind="ExternalInput").ap()
    wb = nc.dram_tensor("wb", [KROWS, npts], BF16_DT, kind="ExternalInput").ap()
    ma = nc.dram_tensor("ma", [KROWS, npts], BF16_DT, kind="ExternalInput").ap()
    o1 = nc.dram_tensor("o1", [P, nt_count], F32, kind="ExternalOutput").ap()
    o2 = nc.dram_tensor("o2", [P, nt_count], F32, kind="ExternalOutput").ap()
    with tile.TileContext(nc) as tc:
        _build_body(tc, (wa, mb_, wb, ma, o1, o2), npts)
    nc.compile()
    return nc


def prep_inputs(xyz1, xyz2, npts=NPTS, nbatch=B):
    """Full inputs -> list of per-core input dicts."""
    in_maps = []
    for b in range(nbatch):
        W1, M1 = _prep_side(np.asarray(xyz1[b]))
        W2, M2 = _prep_side(np.asarray(xyz2[b]))
        in_maps.append({"wa": W1, "mb": M2, "wb": W2, "ma": M1})
    return in_maps


def postprocess(results):
    """Per-core outputs -> scalar chamfer distance."""
    tot1 = tot2 = 0.0
    cnt1 = cnt2 = 0
    for r in results:
        d1 = r["o1"].astype(np.float64)  # [128, nt]; point index = col*128 + row
        d2 = r["o2"].astype(np.float64)
        tot1 += d1.sum()
        tot2 += d2.sum()
        cnt1 += d1.size
        cnt2 += d2.size
    return np.float32(tot1 / cnt1 + tot2 / cnt2)


def kernel(xyz1, xyz2):
    if "nc" not in _cache:
        _cache["nc"] = build_nc()
    nc = _cache["nc"]
    in_maps = prep_inputs(xyz1, xyz2)
    res = run_bass_kernel_spmd(nc, in_maps, list(range(NCORES)))
    return postprocess(res.results)
